# revision 1
# baseline (speedup 1.0000x reference)
"""CosSim-attention kernel for Trainium2 (Bass/Tile), data-parallel over batch.

Problem (full shapes): k (32,256), xs (32,8192,256), mask (32,8192) -> out (32,256)
    k_n   = max(||k||, 1e-8)                      per batch
    xs_n  = max(||xs_s||, 1e-8)                   per position
    cos   = <k, xs_s> / (k_n * xs_n)
    a     = sqrt(cos^2 + 1e-3) * mask + 1e-14
    out   = sum_s a_s * xs_s

Sharding: batch dim 32 -> 8 cores x 4 batches. No cross-core communication.

Per-core layout: each batch's xs (8192,256) is viewed as [128 partitions x
16384 free] so that partition p, free-column f=t*256+j holds xs[p*64+t, j].
The 64-column stat buffers (dots/norms/mask/a) use the matching s = p*64+t
mapping, which makes every DMA fully contiguous per partition.

Engine split per 128-position tile [128,256]:
  - dots  = reduce(xs * k_bcast)      on VectorE  (fused tensor_tensor_reduce)
  - sumsq = reduce(xs^2)              on ScalarE  (activation Square + accum)
  - attn += a_col^T @ xs_tile         on TensorE  (PSUM accumulation over 64 tiles)
k is broadcast to all 128 partitions by a stride-0 DMA so norm/scale ops stay
per-partition.
"""

import numpy as np

import concourse.bacc as bacc
import concourse.bass as bass
import concourse.tile as tile
from concourse import mybir
from concourse.bass_utils import run_bass_kernel_spmd

P = 128            # SBUF partitions
BPC = 4            # batches per core
S = 8192
D = 256
T = S // P         # 64 column-tiles per batch
FB = T * D         # 16384 free elems per partition per batch
CHUNK = 2048       # xs DMA chunk width (1 MiB per transfer)
N_CORES = 8

F32 = mybir.dt.float32
AF = mybir.ActivationFunctionType
ALU = mybir.AluOpType

_NC_CACHE = {}


def build_nc():
    nc = bacc.Bacc("TRN2", debug=False, enable_asserts=False, num_devices=N_CORES)

    k_d = nc.dram_tensor("k", (BPC, D), F32, kind="ExternalInput")
    xs_d = nc.dram_tensor("xs", (BPC, S, D), F32, kind="ExternalInput")
    mask_d = nc.dram_tensor("mask", (BPC, S), F32, kind="ExternalInput")
    out_d = nc.dram_tensor("out", (BPC, D), F32, kind="ExternalOutput")

    k_ap = k_d.ap()
    xs_r = xs_d.ap().rearrange("b (p q) d -> b p (q d)", p=P)   # (4,128,16384)
    mask_r = mask_d.ap().rearrange("b (p t) -> b p t", p=P)     # (4,128,64)
    out_ap = out_d.ap()

    with tile.TileContext(nc) as tc:
        with (
            tc.tile_pool(name="xsp", bufs=2) as xs_pool,
            tc.tile_pool(name="stats", bufs=2) as stats,
            tc.tile_pool(name="scrd", bufs=2) as scr_dve,
            tc.tile_pool(name="scra", bufs=2) as scr_act,
            tc.tile_pool(name="singles", bufs=1) as singles,
            tc.tile_pool(name="outp", bufs=2) as outp,
            tc.tile_pool(name="psum", bufs=2, space="PSUM") as psum,
        ):
            # k broadcast to all partitions: [128, 4, 256] (stride-0 DMA read)
            kbc = singles.tile([P, BPC, D], F32)
            k_bcast_ap = bass.AP(
                tensor=k_ap.tensor, offset=k_ap.offset, ap=[[0, P], *k_ap.ap]
            )
            nc.sync.dma_start(out=kbc, in_=k_bcast_ap)
            kcl = singles.tile([P, BPC], F32)  # clamped ||k||^2, replicated
            soft_eps = singles.tile([P, 1], F32)
            nc.vector.memset(soft_eps, 1e-3)

            for b in range(BPC):
                xs_sb = xs_pool.tile([P, FB], F32, tag="xs_sb")
                for c in range(FB // CHUNK):
                    nc.sync.dma_start(
                        out=xs_sb[:, c * CHUNK:(c + 1) * CHUNK],
                        in_=xs_r[b, :, c * CHUNK:(c + 1) * CHUNK],
                    )
                mask_sb = stats.tile([P, T], F32, tag="mask")
                nc.sync.dma_start(out=mask_sb, in_=mask_r[b])

                # ||k||^2 for this batch, replicated on every partition
                ksq = stats.tile([P, 1], F32, tag="ksq")
                scr0 = scr_dve.tile([P, D], F32, tag="scr_dve")
                nc.vector.scalar_tensor_tensor(
                    out=scr0, in0=kbc[:, b, :], scalar=0.0, in1=kbc[:, b, :],
                    op0=ALU.add, op1=ALU.mult, accum_out=ksq,
                )
                nc.vector.tensor_scalar_max(kcl[:, b:b + 1], ksq, 1e-16)

                dots = stats.tile([P, T], F32, tag="dots")
                sq = stats.tile([P, T], F32, tag="sq")
                for t in range(T):
                    xt = xs_sb[:, t * D:(t + 1) * D]
                    scr = scr_dve.tile([P, D], F32, tag="scr_dve")
                    nc.vector.scalar_tensor_tensor(
                        out=scr, in0=xt, scalar=0.0, in1=kbc[:, b, :],
                        op0=ALU.add, op1=ALU.mult,
                        accum_out=dots[:, t:t + 1],
                    )
                    scra = scr_act.tile([P, D], F32, tag="scr_act")
                    nc.scalar.activation(
                        out=scra, in_=xt, func=AF.Square,
                        accum_out=sq[:, t:t + 1],
                    )

                # per-position weights a = sqrt(dots^2/(ksq*sq) + 1e-3)*mask + 1e-14
                sqc = stats.tile([P, T], F32, tag="sqc")
                nc.vector.tensor_scalar_max(sqc, sq, 1e-16)
                denom = stats.tile([P, T], F32, tag="denom")
                nc.vector.tensor_scalar_mul(denom, sqc, kcl[:, b:b + 1])
                r = stats.tile([P, T], F32, tag="r")
                nc.vector.reciprocal(r, denom)
                d2 = stats.tile([P, T], F32, tag="d2")
                nc.scalar.activation(out=d2, in_=dots, func=AF.Square)
                c2 = stats.tile([P, T], F32, tag="c2")
                nc.vector.tensor_mul(c2, d2, r)
                a0 = stats.tile([P, T], F32, tag="a0")
                nc.scalar.activation(out=a0, in_=c2, func=AF.Sqrt, bias=soft_eps[:, 0:1])
                am = stats.tile([P, T], F32, tag="am")
                nc.vector.tensor_mul(am, a0, mask_sb)
                a = stats.tile([P, T], F32, tag="a")
                nc.vector.tensor_scalar_add(a, am, 1e-14)

                # attn accumulation: out[0, :] += a[:, t]^T @ xs_tile_t
                acc = psum.tile([1, D], F32, tag="acc")
                for t in range(T):
                    nc.tensor.matmul(
                        acc[0:1, :],
                        lhsT=a[:, t:t + 1],
                        rhs=xs_sb[:, t * D:(t + 1) * D],
                        start=(t == 0), stop=(t == T - 1),
                    )
                row = outp.tile([1, D], F32, tag="row")
                nc.scalar.copy(row[0:1, :], acc[0:1, :])
                nc.sync.dma_start(out=out_ap[b:b + 1, :], in_=row[0:1, :])

    nc.compile()
    return nc


def _get_nc():
    if "nc" not in _NC_CACHE:
        _NC_CACHE["nc"] = build_nc()
    return _NC_CACHE["nc"]


def _run(inputs, trace=False):
    k = np.ascontiguousarray(np.asarray(inputs["k"], dtype=np.float32))
    xs = np.ascontiguousarray(np.asarray(inputs["xs"], dtype=np.float32))
    mask = np.ascontiguousarray(np.asarray(inputs["mask"], dtype=np.float32))
    assert k.shape == (32, D) and xs.shape == (32, S, D) and mask.shape == (32, S)

    in_maps = []
    for c in range(N_CORES):
        sl = slice(c * BPC, (c + 1) * BPC)
        in_maps.append({
            "k": np.ascontiguousarray(k[sl]),
            "xs": np.ascontiguousarray(xs[sl]),
            "mask": np.ascontiguousarray(mask[sl]),
        })

    nc = _get_nc()
    res = run_bass_kernel_spmd(
        nc, in_maps, core_ids=list(range(N_CORES)), trace=trace
    )
    out = np.concatenate([r["out"] for r in res.results], axis=0)
    return out, res


def kernel(**inputs):
    out, _ = _run(inputs, trace=False)
    return out

